# revision 10
# baseline (speedup 1.0000x reference)
"""Trainium2 8-core kernel for nn_Actor GNN message passing.

Strategy (see spec sharding_hint): shard nodes (and their incoming edges) by
node-range across 8 cores; replicate the tiny MLP weights. The host does all
data-dependent *layout* work (edge->slot packing, x gathers); the device does
all arithmetic densely:

  - edges are packed into a degree-sorted "field-major" slot layout so the
    per-node segment-sum becomes a short sequence of accumulating identity
    matmuls over contiguous slices (no per-edge gather/scatter on device);
  - hidden layer 1 is one blockdiag matmul per 512-slot chunk (4 node-groups
    x 32 hid = 128 psum partitions, slots on the free dim, bf16 streams);
  - relu+cast is split across ACT and DVE;
  - W2 is folded into the head weights (linearity of segment_sum), so layer 2
    runs in node space, not edge space;
  - final conc normalization (a single global scalar) is applied on host.
"""

import numpy as np
import ml_dtypes

import concourse.bass as bass
from concourse import bacc
import concourse.mybir as mybir
import concourse.tile as tile
from concourse.bass_utils import run_bass_kernel_spmd
from concourse.masks import make_identity

# problem constants (hardcoded per harness contract)
NCORES = 8
N = 250000
E = 4000000
NODE = 4
HID = 32
NN = 50
NF = 3
B = N // NN          # 5000 graphs
NLOC = N // NCORES   # 31250 nodes per core
Q = 4                # partition groups
NQ = 8192            # node columns per group (padded)
SUP = 512            # node columns per superchunk
NSUP = NQ // SUP     # 16
CH = 512             # slot columns per psum chunk

BF = mybir.dt.bfloat16
F32 = mybir.dt.float32
bf16 = ml_dtypes.bfloat16

LAST_EXEC_NS = None
LAST_RESULTS = None


def _host_prep(x, edge_index, edge_attr):
    """Pure index/layout preprocessing. No arithmetic on tensor values."""
    row = np.ascontiguousarray(edge_index[0]).astype(np.int64)
    col = np.ascontiguousarray(edge_index[1]).astype(np.int64)
    attr = np.ascontiguousarray(edge_attr[:, 0])

    deg_full = np.bincount(row, minlength=N).astype(np.int64)

    # per-core rank assignment: sort local nodes by degree desc, deal
    # round-robin into 4 groups -> each group's degree sequence is desc.
    rank = np.empty(N, np.int64)          # node -> rank within its core
    node_at = np.full((NCORES, Q, NQ), -1, np.int64)
    degs_q = np.zeros((NCORES, Q, NQ), np.int64)
    for c in range(NCORES):
        lo, hi = c * NLOC, (c + 1) * NLOC
        dl = deg_full[lo:hi]
        order = np.argsort(-dl, kind="stable")
        r = np.empty(NLOC, np.int64)
        r[order] = np.arange(NLOC)
        rank[lo:hi] = r
        g_of = np.arange(NLOC) % Q
        j_of = np.arange(NLOC) // Q
        node_at[c, g_of, j_of] = lo + order
        degs_q[c, g_of, j_of] = dl[order]

    D = degs_q.max(axis=(0, 1))            # common profile [NQ], desc
    DMAX = int(D[0])

    # per-superchunk field structure
    m = np.zeros((NSUP, DMAX), np.int64)   # m[s,k] = cols in field k
    F = np.zeros((NSUP, DMAX), np.int64)   # field offsets within superchunk
    SB = np.zeros(NSUP + 1, np.int64)      # superchunk slot base
    dsup_max = np.zeros(NSUP, np.int64)
    for s in range(NSUP):
        Dsub = D[s * SUP:(s + 1) * SUP]
        dm = int(Dsub[0])
        dsup_max[s] = dm
        for k in range(dm):
            m[s, k] = int((Dsub > k).sum())
        F[s, :dm] = np.r_[0, np.cumsum(m[s, :dm - 1])] if dm > 0 else 0
        SB[s + 1] = SB[s] + m[s, :dm].sum()
    S_q = int(SB[NSUP])

    # per-edge slot positions
    core_e = row // NLOC
    r_e = rank[row]
    g_e = r_e % Q
    j_e = r_e // Q
    s_e = j_e // SUP
    jj_e = j_e - s_e * SUP
    # within-node edge counter k
    order_e = np.argsort(row, kind="stable")
    starts = np.r_[0, np.cumsum(deg_full)[:-1]]
    k_sorted = np.arange(E) - starts[row[order_e]]
    k_e = np.empty(E, np.int64)
    k_e[order_e] = k_sorted
    pos_e = SB[s_e] + F[s_e, k_e] + jj_e   # free index in xs

    # build packed per-core tensors
    xs = np.zeros((NCORES, 36, S_q), np.float32)
    feats = np.concatenate([x[row], x[col], attr[:, None]], axis=1)  # [E,9]
    for g in range(Q):
        msk = g_e == g
        ce, pe = core_e[msk], pos_e[msk]
        fe = feats[msk]
        for f in range(9):
            xs[ce, 9 * g + f, pe] = fe[:, f]

    valid = node_at >= 0
    nsafe = np.where(valid, node_at, 0)
    xn = np.zeros((NCORES, 16, NQ), np.float32)
    for g in range(Q):
        for f in range(NODE):
            xn[:, 4 * g + f, :] = np.where(valid[:, g, :], x[nsafe[:, g, :], f], 0.0)

    dg = np.where(valid, degs_q, 0).astype(np.float32)            # [NC,Q,NQ]
    pcn = -(np.where(valid, D[None, None, :] - degs_q, 0)).astype(np.float32)
    mk = valid.astype(np.float32)
    # high expanded: tail positions (graph pos 47,48,49) get high[j], else 0
    posn = nsafe % NN
    hx_idx = np.clip(posn - (NN - NF), 0, NF - 1)
    return dict(
        xs=xs, xn=xn, dg=dg, pcn=pcn, mk=mk,
        valid=valid, nsafe=nsafe, posn=posn, hx_idx=hx_idx,
        S_q=S_q, m=m, F=F, SB=SB, dsup_max=dsup_max, DMAX=DMAX,
        node_at=node_at,
    )


def _build_program(meta, use_b1, use_b2):
    S_q = meta["S_q"]
    m, F, SB, dsup_max = meta["m"], meta["F"], meta["SB"], meta["dsup_max"]
    slots_max = int(max(int(SB[s + 1] - SB[s]) for s in range(NSUP)) or 1)

    nc = bacc.Bacc()
    # inputs
    t_xs = nc.declare_dram_parameter("xs", [36, S_q], BF, isOutput=False)
    t_xn = nc.declare_dram_parameter("xn", [16, NQ], BF, isOutput=False)
    t_dg = nc.declare_dram_parameter("dg", [4, NQ], BF, isOutput=False)
    t_pcn = nc.declare_dram_parameter("pcn", [4, NQ], BF, isOutput=False)
    t_mk = nc.declare_dram_parameter("mk", [128, NQ // 32], F32, isOutput=False)
    t_hx = nc.declare_dram_parameter("hx", [128, NQ // 32], F32, isOutput=False)
    t_w1blk = nc.declare_dram_parameter("w1blk", [36, 128], F32, isOutput=False)
    t_b1t = nc.declare_dram_parameter("b1t", [128, 1], F32, isOutput=False)
    t_whx = nc.declare_dram_parameter("whx", [16, 96], F32, isOutput=False)
    t_wha = nc.declare_dram_parameter("wha", [32, 3], F32, isOutput=False)
    t_w2T = nc.declare_dram_parameter("w2T", [32, 32], F32, isOutput=False)
    t_b2c = nc.declare_dram_parameter("b2c", [32, 1], F32, isOutput=False)
    t_b1c = nc.declare_dram_parameter("b1c", [32, 1], F32, isOutput=False)
    t_hb = nc.declare_dram_parameter("hb", [96, 1], F32, isOutput=False)
    # outputs
    o_conc = nc.declare_dram_parameter("out_conc", [4, NQ], F32, isOutput=True)
    o_ord = nc.declare_dram_parameter("out_ord", [128, NQ // 32], F32, isOutput=True)
    o_s = nc.declare_dram_parameter("out_s", [128, 1], F32, isOutput=True)

    with tile.TileContext(nc) as tc:
        with tc.tile_pool(name="const", bufs=1) as cp, \
             tc.tile_pool(name="cpsum", bufs=1, space="PSUM") as cpp:
            # ---- preamble: load + cast weights ----
            w1_f = cp.tile([36, 128], F32, tag="w1f")
            nc.sync.dma_start(out=w1_f[:], in_=t_w1blk[:])
            w1b = cp.tile([36, 128], BF, tag="w1b")
            nc.vector.tensor_copy(out=w1b[:], in_=w1_f[:])

            b1t = cp.tile([128, 1], F32, tag="b1t")
            nc.sync.dma_start(out=b1t[:], in_=t_b1t[:])

            whx_f = cp.tile([16, 96], F32, tag="whxf")
            nc.sync.dma_start(out=whx_f[:], in_=t_whx[:])
            whx = cp.tile([16, 96], BF, tag="whx")
            nc.vector.tensor_copy(out=whx[:], in_=whx_f[:])

            wha_f = cp.tile([32, 3], F32, tag="whaf")
            nc.sync.dma_start(out=wha_f[:], in_=t_wha[:])
            wha_b = cp.tile([32, 3], BF, tag="whab")
            nc.vector.tensor_copy(out=wha_b[:], in_=wha_f[:])

            w2T_f = cp.tile([32, 32], F32, tag="w2tf")
            nc.sync.dma_start(out=w2T_f[:], in_=t_w2T[:])
            w2T_b = cp.tile([32, 32], BF, tag="w2tb")
            nc.vector.tensor_copy(out=w2T_b[:], in_=w2T_f[:])

            hb0 = cp.tile([96, 1], F32, tag="hb0")
            nc.sync.dma_start(out=hb0[:], in_=t_hb[:])

            ident_f = cp.tile([128, 128], F32, tag="idf")
            make_identity(nc, ident_f[:])
            ident = cp.tile([128, 128], BF, tag="idb")
            nc.vector.tensor_copy(out=ident[:], in_=ident_f[:])

            # Wha2 = W2 @ Wh_a  -> [32,3]
            wha2_p = cpp.tile([32, 4], F32, tag="wha2p")
            nc.tensor.matmul(out=wha2_p[:, :3], lhsT=w2T_b[:], rhs=wha_b[:],
                             start=True, stop=True)
            wha2_b = cp.tile([32, 3], BF, tag="wha2b")
            nc.vector.tensor_copy(out=wha2_b[:], in_=wha2_p[:, :3])
            # blockdiag [128,12] of Wha2 per group
            wha_blk = cp.tile([128, 96], BF, tag="whablk")
            nc.vector.memset(wha_blk[:], 0.0)
            for g in range(Q):
                for h in range(3):
                    nc.vector.tensor_copy(
                        out=wha_blk[32 * g:32 * g + 32, 32 * h + g:32 * h + g + 1],
                        in_=wha2_b[:, h:h + 1])

            degl = pcl = None
            if use_b2:
                b2c_f = cp.tile([32, 1], F32, tag="b2cf")
                nc.sync.dma_start(out=b2c_f[:], in_=t_b2c[:])
                b2c_b = cp.tile([32, 1], BF, tag="b2cb")
                nc.vector.tensor_copy(out=b2c_b[:], in_=b2c_f[:])
                sb2_p = cpp.tile([1, 4], F32, tag="sb2p")
                nc.tensor.matmul(out=sb2_p[:, :3], lhsT=b2c_b[:], rhs=wha_b[:],
                                 start=True, stop=True)
                sb2_b = cp.tile([1, 3], BF, tag="sb2b")
                nc.vector.tensor_copy(out=sb2_b[:], in_=sb2_p[:, :3])
                sb2_d = nc.dram_tensor("sb2_d", [1, 3], BF)
                nc.sync.dma_start(out=sb2_d[:], in_=sb2_b[:])
                degl = cp.tile([4, 96], BF, tag="degl")
                nc.vector.memset(degl[:], 0.0)
                for g in range(Q):
                    for h in range(3):
                        nc.sync.dma_start(
                            out=degl[g:g + 1, 32 * h + g:32 * h + g + 1],
                            in_=sb2_d[:, h:h + 1])
            if use_b1:
                b1c_f = cp.tile([32, 1], F32, tag="b1cf")
                nc.sync.dma_start(out=b1c_f[:], in_=t_b1c[:])
                rb1_b = cp.tile([32, 1], BF, tag="rb1b")
                nc.scalar.activation(out=rb1_b[:], in_=b1c_f[:],
                                     func=mybir.ActivationFunctionType.Relu)
                sc2_p = cpp.tile([1, 4], F32, tag="sc2p")
                nc.tensor.matmul(out=sc2_p[:, :3], lhsT=rb1_b[:], rhs=wha2_b[:],
                                 start=True, stop=True)
                sc2_b = cp.tile([1, 3], BF, tag="sc2b")
                nc.vector.tensor_copy(out=sc2_b[:], in_=sc2_p[:, :3])
                sc2_d = nc.dram_tensor("sc2_d", [1, 3], BF)
                nc.sync.dma_start(out=sc2_d[:], in_=sc2_b[:])
                pcl = cp.tile([4, 96], BF, tag="pcl")
                nc.vector.memset(pcl[:], 0.0)
                for g in range(Q):
                    for h in range(3):
                        nc.sync.dma_start(
                            out=pcl[g:g + 1, 32 * h + g:32 * h + g + 1],
                            in_=sc2_d[:, h:h + 1])

            # head bias: hb0 + inside-offsets (in-place slice adds keep the
            # per-instruction sync-wait count low)
            hbias = cp.tile([96, 1], F32, tag="hbias")
            nc.vector.tensor_copy(out=hbias[:], in_=hb0[:])
            nc.vector.tensor_scalar_add(out=hbias[0:4, :], in0=hbias[0:4, :],
                                        scalar1=1e-10)
            nc.vector.tensor_scalar_add(out=hbias[32:36, :], in0=hbias[32:36, :],
                                        scalar1=1e-20)
            nc.vector.tensor_scalar_add(out=hbias[64:68, :], in0=hbias[64:68, :],
                                        scalar1=1e-20)

            aggT = cp.tile([128, NQ], BF, tag="aggT")

            # ---- edge phase ----
            with tc.tile_pool(name="xsp", bufs=2) as xsp, \
                 tc.tile_pool(name="rp", bufs=2) as rp, \
                 tc.tile_pool(name="hps", bufs=4, space="PSUM") as hps, \
                 tc.tile_pool(name="aps", bufs=2, space="PSUM") as aps:
                relu_cnt = 0
                for s in range(NSUP):
                    slots = int(SB[s + 1] - SB[s])
                    dm = int(dsup_max[s])
                    m0 = int(m[s, 0]) if dm > 0 else 0
                    if slots > 0:
                        xs_t = xsp.tile([36, slots_max], BF, tag="xs")
                        nc.sync.dma_start(out=xs_t[:, :slots],
                                          in_=t_xs[:, int(SB[s]):int(SB[s + 1])])
                        r_t = rp.tile([128, slots_max], BF, tag="r")
                        nchunk = (slots + CH - 1) // CH
                        for ci in range(nchunk):
                            a = ci * CH
                            b = min(slots, a + CH)
                            hp = hps.tile([128, CH], F32, tag="h")
                            nc.tensor.matmul(out=hp[:, :b - a], lhsT=w1b[:],
                                             rhs=xs_t[:, a:b], start=True, stop=True)
                            # relu + bias + cast, alternate ACT(5)/DVE(4)
                            if relu_cnt % 9 < 5:
                                nc.scalar.activation(
                                    out=r_t[:, a:b], in_=hp[:, :b - a],
                                    func=mybir.ActivationFunctionType.Relu,
                                    bias=b1t[:] if use_b1 else 0.0)
                            else:
                                if use_b1:
                                    nc.vector.tensor_scalar(
                                        out=r_t[:, a:b], in0=hp[:, :b - a],
                                        scalar1=b1t[:], scalar2=0.0,
                                        op0=mybir.AluOpType.add,
                                        op1=mybir.AluOpType.max)
                                else:
                                    nc.vector.tensor_scalar_max(
                                        out=r_t[:, a:b], in0=hp[:, :b - a],
                                        scalar1=0.0)
                            relu_cnt += 1
                        # segment-sum via accumulating identity matmuls
                        ap_t = aps.tile([128, SUP], F32, tag="agg")
                        for k in range(dm):
                            mk_ = int(m[s, k])
                            fk = int(F[s, k])
                            nc.tensor.matmul(out=ap_t[:, :mk_], lhsT=ident[:],
                                             rhs=r_t[:, fk:fk + mk_],
                                             start=(k == 0), stop=(k == dm - 1))
                        nc.vector.tensor_copy(out=aggT[:, s * SUP:s * SUP + m0],
                                              in_=ap_t[:, :m0])
                    if m0 < SUP:
                        nc.vector.memset(aggT[:, s * SUP + m0:(s + 1) * SUP], 0.0)

            # ---- head phase ----
            with tc.tile_pool(name="hd", bufs=1) as hd, \
                 tc.tile_pool(name="hdps", bufs=2, space="PSUM") as hdps:
                xn_b = hd.tile([16, NQ], BF, tag="xnb")
                nc.sync.dma_start(out=xn_b[:], in_=t_xn[:])
                sp = hd.tile([96, NQ], F32, tag="sp")
                dg_b = pcn_b = None
                if use_b2:
                    dg_b = hd.tile([4, NQ], BF, tag="dgb")
                    nc.sync.dma_start(out=dg_b[:], in_=t_dg[:])
                if use_b1:
                    pcn_b = hd.tile([4, NQ], BF, tag="pcnb")
                    nc.sync.dma_start(out=pcn_b[:], in_=t_pcn[:])
                for t in range(NSUP):
                    sl = slice(t * SUP, (t + 1) * SUP)
                    php = hdps.tile([96, SUP], F32, tag="ph")
                    nc.tensor.matmul(out=php[:], lhsT=whx[:], rhs=xn_b[:, sl],
                                     start=True, stop=not (use_b2 or use_b1))
                    nc.tensor.matmul(out=php[:], lhsT=wha_blk[:], rhs=aggT[:, sl],
                                     start=False, stop=not (use_b2 or use_b1))
                    if use_b2:
                        nc.tensor.matmul(out=php[:], lhsT=degl[:], rhs=dg_b[:, sl],
                                         start=False, stop=not use_b1)
                    if use_b1:
                        nc.tensor.matmul(out=php[:], lhsT=pcl[:], rhs=pcn_b[:, sl],
                                         start=False, stop=True)
                    # softplus(z) = ln(1 + exp(z)); z in [-35, 50] so exp is
                    # f32-safe (walrus lacks a named softplus ACT function)
                    eT = hd.tile([96, SUP], F32, tag="eT")
                    nc.scalar.activation(out=eT[:], in_=php[:],
                                         func=mybir.ActivationFunctionType.Exp,
                                         bias=hbias[:])
                    nc.scalar.activation(out=sp[:, sl], in_=eT[:],
                                         func=mybir.ActivationFunctionType.Ln,
                                         bias=1.0)

                # Repack 4-partition head rows into dense [128, 256] tiles
                # (full DVE lane width; 2-input ops need equal base partitions).
                NP = NQ // 32
                def packed_view(src_rows):
                    # [4, NQ] rows of sp -> iteration order (g, b, j)
                    return src_rows.rearrange("g (b j) -> g b j", b=32)

                concP = hd.tile([128, NP], F32, tag="concP")
                alP = hd.tile([128, NP], F32, tag="alP")
                beP = hd.tile([128, NP], F32, tag="beP")
                nc.sync.dma_start(out=concP[:], in_=packed_view(sp[0:4, :]))
                nc.sync.dma_start(out=alP[:], in_=packed_view(sp[32:36, :]))
                nc.sync.dma_start(out=beP[:], in_=packed_view(sp[64:68, :]))
                nc.sync.dma_start(out=o_conc[:], in_=sp[0:4, :])

                mkP = hd.tile([128, NP], F32, tag="mkP")
                nc.sync.dma_start(out=mkP[:], in_=t_mk[:])
                cm = hd.tile([128, NP], F32, tag="cm")
                nc.vector.tensor_mul(out=cm[:], in0=concP[:], in1=mkP[:])
                s_red = hd.tile([128, 1], F32, tag="sred")
                nc.vector.tensor_reduce(out=s_red[:], in_=cm[:],
                                        axis=mybir.AxisListType.X,
                                        op=mybir.AluOpType.add)
                nc.sync.dma_start(out=o_s[:], in_=s_red[:])

                # order = (alpha+eps)/(alpha+beta+2eps) * high
                nc.vector.tensor_scalar_add(out=alP[:], in0=alP[:], scalar1=1e-20)
                nc.vector.tensor_scalar_add(out=beP[:], in0=beP[:], scalar1=1e-20)
                tP = hd.tile([128, NP], F32, tag="tP")
                nc.vector.tensor_add(out=tP[:], in0=alP[:], in1=beP[:])
                rP = hd.tile([128, NP], F32, tag="rP")
                nc.vector.reciprocal(out=rP[:], in_=tP[:])
                nc.vector.tensor_mul(out=alP[:], in0=alP[:], in1=rP[:])
                hxP = hd.tile([128, NP], F32, tag="hxP")
                nc.sync.dma_start(out=hxP[:], in_=t_hx[:])
                nc.vector.tensor_mul(out=alP[:], in0=alP[:], in1=hxP[:])
                nc.sync.dma_start(out=o_ord[:], in_=alP[:])

    if not nc.is_finalized():
        nc.finalize()
    return nc


def kernel(**inputs):
    global LAST_EXEC_NS, LAST_RESULTS
    x = np.asarray(inputs["x"], np.float32)
    edge_index = np.asarray(inputs["edge_index"])
    edge_attr = np.asarray(inputs["edge_attr"], np.float32)
    high = np.asarray(inputs["high"], np.float32)
    W1 = np.asarray(inputs["W1"], np.float32)
    b1 = np.asarray(inputs["b1"], np.float32)
    W2 = np.asarray(inputs["W2"], np.float32)
    b2 = np.asarray(inputs["b2"], np.float32)
    Wc = np.asarray(inputs["Wc"], np.float32)
    bc = np.asarray(inputs["bc"], np.float32)
    Wmu = np.asarray(inputs["Wmu"], np.float32)
    bmu = np.asarray(inputs["bmu"], np.float32)
    Wsig = np.asarray(inputs["Wsig"], np.float32)
    bsig = np.asarray(inputs["bsig"], np.float32)

    meta = _host_prep(x, edge_index, edge_attr)
    use_b1 = bool(np.any(b1 != 0))
    use_b2 = bool(np.any(b2 != 0))

    # weight layout tensors (pure placement)
    w1blk = np.zeros((36, 128), np.float32)
    for g in range(Q):
        w1blk[9 * g:9 * g + 9, 32 * g:32 * g + 32] = W1
    b1t = np.tile(b1, Q)[:, None].astype(np.float32)
    heads_x = [Wc[:4, 0], Wmu[:4, 0], Wsig[:4, 0]]
    whx = np.zeros((16, 96), np.float32)
    for g in range(Q):
        for h in range(3):
            whx[4 * g:4 * g + 4, 32 * h + g] = heads_x[h]
    wha = np.stack([Wc[4:, 0], Wmu[4:, 0], Wsig[4:, 0]], axis=1).astype(np.float32)
    hb = np.zeros((96, 1), np.float32)
    for h, bv in enumerate([bc[0], bmu[0], bsig[0]]):
        hb[32 * h:32 * h + 4, 0] = bv

    # high expanded per node column
    hx = np.where(meta["posn"] >= NN - NF, high[meta["hx_idx"]], 0.0)
    hx = np.where(meta["valid"], hx, 0.0).astype(np.float32)

    nc = _build_program(meta, use_b1, use_b2)

    in_maps = []
    for c in range(NCORES):
        in_maps.append(dict(
            xs=meta["xs"][c].astype(bf16),
            xn=meta["xn"][c].astype(bf16),
            dg=meta["dg"][c].astype(bf16),
            pcn=meta["pcn"][c].astype(bf16),
            mk=meta["mk"][c].reshape(Q, 32, NQ // 32).reshape(128, NQ // 32).astype(np.float32),
            hx=hx[c].reshape(Q, 32, NQ // 32).reshape(128, NQ // 32).astype(np.float32),
            w1blk=w1blk, b1t=b1t, whx=whx, wha=wha,
            w2T=np.ascontiguousarray(W2.T),
            b2c=b2[:, None].astype(np.float32),
            b1c=b1[:, None].astype(np.float32),
            hb=hb,
        ))

    import os
    trace = bool(os.environ.get("BASS_KERNEL_TRACE"))
    res = run_bass_kernel_spmd(nc, in_maps, list(range(NCORES)), trace=trace)
    LAST_EXEC_NS = res.exec_time_ns if trace else None
    LAST_RESULTS = res

    # ---- host assembly: unpermute + global normalize ----
    S = 1e-20
    conc_full = np.zeros(N, np.float32)
    ord_full = np.zeros(N, np.float32)
    valid, node_at = meta["valid"], meta["node_at"]
    for c in range(NCORES):
        oc = np.asarray(res.results[c]["out_conc"])
        oo = np.asarray(res.results[c]["out_ord"]).reshape(Q, 32, NQ // 32).reshape(Q, NQ)
        os_ = np.asarray(res.results[c]["out_s"])
        S += float(os_.sum())
        v = valid[c]
        conc_full[node_at[c][v]] = oc[v]
        ord_full[node_at[c][v]] = oo[v]

    inv = (conc_full / S).reshape(B, NN)
    order = ord_full.reshape(B, NN)[:, NN - NF:]
    return np.concatenate([inv, order], axis=1).astype(np.float32)
